# revision 76
# baseline (speedup 1.0000x reference)
"""Multi-head attention layer (B=8, N=1024, E=1024, H=16, D=64) on 8 TRN2
NeuronCores, data-parallel over batch (one batch element per core, weights
replicated, no collectives).

V14 schedule (V12 + double-slots + restructured prologue + V-in-loop):
- Prologue: V-projection split into per-nh chains (first 8 chains need only
  the nh0 half of wvb + one x chunk, so PE starts after ~1.25MB of DMA);
  x rides the sync HW ring in 8 chunks, weights ride the scalar HW ring in
  first-use order (fine wvb chunks first; gpsimd DMA is a ~10x slower
  software DGE, used only for the bias broadcast). The two prologue qkT
  tiles interleave with the last 8 V chains.
- Attention loop: 4 double-slots per half — TWO adjacent score pairs (the
  full-row <-> row-tiled LDWEIGHTS transition tax is paid once per 2 slots;
  the second pair enters at +222ns instead of +311ns), then 4 AV + 4 qkT
  filler matmuls. Score psum pool has 3 bufs (6 PSUM banks) so adjacent
  pairs never wait on the exp rotation; qkT/AV accumulators get 1 bank
  each. Per-half qkT thunk batches (never straddling the DVE-heavy half
  boundaries, which triggers HAM re-throttle cascades).
- Softmax normalization: ones-column in v_sb yields row sums with the AV
  matmul; reciprocal on DVE, broadcast via DRAM-bounce gpsimd DMA, applied
  by deferred DVE muls two halves later.
- The loop is PE-paced at ~1190ns/slot, ~2% above the ScalarE exp floor
  (1114ns ACTIVATE + ~53ns queue overhead); exp is the only engine that
  can do exp, so ~143us of the runtime is irreducible on this
  architecture. FP8/DoubleRow was measured numerically and rejected: every
  e4m3 quantization config lands at or above the 2e-2 rel-err gate
  (qk-path 7.9e-2, v-path 2.0e-2, av-path 2.0e-2, out-path 2.5e-2).

- V-in-loop: the last two nh1 V chains ride half (0,0)'s filler slack
  (2/sub-slot, accumulating in the pat pool which is idle until (0,1);
  nh1 v_sb tiles are first consumed by head-pair 4 at half (4,1)), and
  the prologue qkT tiles ride V chains 3+ (their nh0 thunks need only the
  first half of x). hs1 AV chains are slot-gated so the hs0 staged-copy
  (single pat buffer) always lands first — otherwise the PE idles
  1.5-4us mid-half and HAM sometimes re-throttles on top.

- V16/V17: fin muls were reaching the strict-FIFO DVE queue head before
  their gpsimd broadcast landed, jamming the qkT bias-add behind them
  (PE stall -> HAM re-throttle cascades). Fixes: recip/broadcast path in
  bf16 (halves broadcast bytes; +0.1e-3 rel err), all broadcasts on the
  sync HW ring (~0.5us vs multi-us gpsimd software DGE), fins deferred
  one extra half for hp<6, staged=7/bcast=7 bufs (recip tiles at bufs=1
  to fund them).

Measured (nominal device clocks): 242.7-245.3us, rel err 2.5e-3;
run-to-run device noise is +-1.5us. (V12 was 250.7-254.6us. Runs on a
throttled device scale ~clock; the device occasionally throttles ~20%
for a few minutes.)
"""

import os

import ml_dtypes
import numpy as np

B, N, E, H, D = 8, 1024, 1024, 16, 64
P = 128
KE = E // P
NT = N // P
MQK = 2 * E // P
DP1 = D + 1

TRACE = os.environ.get("BASS_KERNEL_TRACE", "0") == "1"
LAST_EXEC_NS = None
LAST_RESULT = None

_COMPILED = None


def _build():
    import concourse.bass as bass
    import concourse.tile as tile
    from concourse import bacc, mybir

    f32 = mybir.dt.float32
    bf16 = mybir.dt.bfloat16
    AF = mybir.ActivationFunctionType
    MS = bass.MemorySpace

    nc = bacc.Bacc(
        "TRN2", target_bir_lowering=False, debug=False, enable_asserts=True
    )

    xb_d = nc.dram_tensor("xb", [P, KE * N], bf16, kind="ExternalInput")
    wqb_d = nc.dram_tensor("wqb", [P, KE * 2 * E], bf16, kind="ExternalInput")
    qkb_d = nc.dram_tensor("qkb", [P, MQK], f32, kind="ExternalInput")
    wvb_d = nc.dram_tensor("wvb", [P, KE * E], bf16, kind="ExternalInput")
    wob_d = nc.dram_tensor("wob", [P, KE * E], bf16, kind="ExternalInput")
    ob_d = nc.dram_tensor("ob", [1, E], f32, kind="ExternalInput")
    y_d = nc.dram_tensor("y", [N, E], f32, kind="ExternalOutput")

    with tile.TileContext(nc) as tc, tc.tile_pool(name="persist", bufs=1) as persist:
        qkT = [
            persist.tile([P, N], bf16, tag=f"qkT{m}", name=f"qkT{m}")
            for m in range(MQK)
        ]
        v_sb = [
            persist.tile([P, H * DP1], bf16, tag=f"v{m}", name=f"v{m}")
            for m in range(NT)
        ]
        attnT = [
            [
                persist.tile([P, 512], bf16, tag=f"attnT{k}_{ih}", name=f"attnT{k}_{ih}")
                for ih in range(2)
            ]
            for k in range(KE)
        ]
        woa = persist.tile([P, KE * E], bf16, tag="woa", name="woa")
        qkb_sb = persist.tile([P, MQK], f32, tag="qkb", name="qkb_sb")
        bias_bc = persist.tile([P, E], f32, tag="bias_bc", name="bias_bc")

        with (
            tc.tile_pool(name="ldx", bufs=1) as ldx,
            tc.tile_pool(name="ldqk", bufs=1) as ldqk,
            tc.tile_pool(name="psum", bufs=3, space=MS.PSUM) as psum_pool,
            tc.tile_pool(name="psqk", bufs=1, space=MS.PSUM) as psqk_pool,
            tc.tile_pool(name="pat", bufs=1, space=MS.PSUM) as pat_pool,
        ):
            xa = ldx.tile([P, KE * N], bf16, tag="xa", name="xa")
            wqa = ldqk.tile([P, KE * 2 * E], bf16, tag="wqa", name="wqa")
            xtok = xa[:].rearrange("p (m c) -> p m c", c=N)

            def qk_thunks(m, nhs=(0, 1)):
                """16 matmul thunks computing qkT[m] in two [128,512]
                halves from the dedicated psqk pool; each half's DVE
                bias-add is issued with its 8th matmul."""
                state = {}

                def mk(nh, k):
                    def t():
                        if k == 0:
                            state[nh] = psqk_pool.tile(
                                [P, 512], f32, tag="qkh", name="psqh"
                            )
                        ps = state[nh]
                        nc.tensor.matmul(
                            ps[:],
                            wqa[:, m * N + k * P : m * N + (k + 1) * P],
                            xtok[:, 4 * nh : 4 * nh + 4, k * P : (k + 1) * P],
                            start=(k == 0),
                            stop=(k == KE - 1),
                        )
                        if k == KE - 1:
                            nc.vector.tensor_scalar_add(
                                qkT[m][:, nh * 512 : (nh + 1) * 512],
                                ps[:],
                                qkb_sb[:, m : m + 1],
                            )

                    return t

                return [(0, mk(nh, k)) for nh in nhs for k in range(KE)]

            # wva nh1 half outlives the prologue: its last 3 V chains ride
            # half (0,0)'s filler slack (nh1 v_sb tiles are first consumed
            # by head-pair 4 at half (4,1), ~100us into the loop)
            wva1 = ldqk.tile([P, 4 * E], bf16, tag="wva1", name="wva1")

            # ---------------- stage 2 first: v (needs only xb + wvb) --------
            with tc.tile_pool(name="ldv", bufs=1) as ldv:
                wva = ldv.tile([P, 4 * E], bf16, tag="wva", name="wva")

                # Concurrent DMAs share bandwidth fairly (no issue-order
                # priority) but each queue drains serially, so spread the
                # loads across queues ordered by first-use time: V needs
                # wvb-nh0 + x[0] first (~1.25MB critical mass), the
                # prologue qkT tiles need wqa slices m=0/m=8 + qkb by
                # ~15us, wqa-rest by the loop, woa/ob only at out_proj.
                # x rides the sync HW ring, weight loads ride the scalar HW
                # ring in first-use order (the two rings share HBM read
                # bandwidth, so splitting one tensor across both doesn't
                # help; gpsimd DMA is a ~10x slower software DGE — only the
                # latency-tolerant bias broadcast goes there).
                for m in range(NT):
                    nc.sync.dma_start(
                        xa[:, m * N : (m + 1) * N], xb_d[:, m * N : (m + 1) * N]
                    )
                # wvb nh0 in fine chunks: dependency tracking is per-DMA, so
                # smaller pieces let the first V chain start ~4us earlier
                for q in range(4):
                    nc.scalar.dma_start(
                        wva[:, q * E : (q + 1) * E], wvb_d[:, q * E : (q + 1) * E]
                    )
                nc.scalar.dma_start(wqa[:, 0:N], wqb_d[:, 0:N])
                nc.scalar.dma_start(wqa[:, 8 * N : 9 * N], wqb_d[:, 8 * N : 9 * N])
                nc.scalar.dma_start(qkb_sb[:], qkb_d[:, :])
                nc.scalar.dma_start(wva1[:, 0 : 2 * E], wvb_d[:, 4 * E : 6 * E])
                nc.scalar.dma_start(wva1[:, 2 * E : 4 * E], wvb_d[:, 6 * E : 8 * E])
                nc.scalar.dma_start(wqa[:, N : 8 * N], wqb_d[:, N : 8 * N])
                nc.scalar.dma_start(wqa[:, 9 * N : 16 * N], wqb_d[:, 9 * N : 16 * N])
                # woa isn't needed until out_proj (~210us): park it on the
                # slow-but-now-idle gpsimd DGE (~2MB arrives ~100us in),
                # freeing 2MB of early scalar-ring bandwidth for the
                # prologue-critical loads
                nc.gpsimd.dma_start(woa[:, :], wob_d[:, :])
                nc.gpsimd.dma_start(bias_bc[:], ob_d[0:1, :].to_broadcast((P, E)))

                def v_chain(m, nh):
                    """Half a V m-tile: heads nh*8..nh*8+7, needs only the
                    nh half of wvb (1MB) + x chunk m."""
                    ps = psum_pool.tile([P, N], f32, tag="big", name="psv")
                    wsrc = wva if nh == 0 else wva1
                    for k in range(KE):
                        nc.tensor.matmul(
                            ps[:, 0:512],
                            xa[:, m * N + k * P : m * N + (k + 1) * P],
                            wsrc[:, k * 512 : (k + 1) * 512],
                            start=(k == 0),
                            stop=(k == KE - 1),
                        )
                    src3 = ps[:, 0:512].rearrange("p (h c) -> p h c", c=D)
                    dst3 = v_sb[m][:].rearrange("p (h c) -> p h c", c=DP1)
                    # copy on DVE: keeps the PSUM release prompt (the Act
                    # sequencer is busy with DMA descriptor-gen here)
                    nc.vector.tensor_copy(dst3[:, 8 * nh : 8 * nh + 8, 0:D], src3)
                    nc.vector.memset(dst3[:, 8 * nh : 8 * nh + 8, D : D + 1], 1.0)

                # nh-outer so the first 8 chains need only half of wvb; the
                # two prologue qkT tiles ride chains 3+ (their nh0 thunks
                # need only the first half of x, which has landed by then).
                # The last 3 nh1 chains ride half (0,0) in the loop.
                pro_qk = qk_thunks(0) + qk_thunks(8)
                chains = [(m, 0) for m in range(NT)] + [(m, 1) for m in range(6)]
                for ci, (m, nh) in enumerate(chains):
                    v_chain(m, nh)
                    if ci >= 3:
                        for _ in range(4):
                            if pro_qk:
                                pro_qk.pop(0)[1]()
                while pro_qk:
                    pro_qk.pop(0)[1]()

            # ---------------- interleaved qkT + attention -------------------
            with (
                tc.tile_pool(name="probs", bufs=16) as probs_pool,
                tc.tile_pool(name="staged", bufs=7) as staged_pool,
                tc.tile_pool(name="bcast", bufs=7) as bcast_pool,
                tc.tile_pool(name="sums", bufs=2) as sums_pool,
                tc.tile_pool(name="ysb", bufs=3) as y_pool,
                tc.tile_pool(name="dram", bufs=4, space=MS.DRAM) as dram_pool,
            ):

                def av_thunks(hp, ih, pts, fin_box, self_mode=False):
                    """16 (min_slot, thunk) AV matmuls; staging/reciprocal/
                    broadcast issue inline with the 8th matmul of each head.
                    On completion, (hp, ih, stg, bcs) goes to fin_box. In
                    self_mode the thunks consume the half's own pts list, so
                    thunk (hs, jt) may only run at slot >= jt."""
                    state = {"stg": [], "bcs": []}
                    # recip/broadcast path in bf16: ~0.3% extra error on the
                    # normalization factor (well under the 2e-2 gate), halves
                    # the broadcast bytes, and makes the hp=7 sync-ring
                    # broadcasts dtype-clean (HW DGE can't convert)
                    rd = dram_pool.tile(
                        [2, 512], bf16, tag="recip_dram", name="recip_dram"
                    )

                    def mk(hs, jt):
                        def t():
                            if jt == 0:
                                state[hs] = pat_pool.tile(
                                    [DP1, 512], f32, tag="pat", name="pat"
                                )
                            pa = state[hs]
                            h = 2 * hp + hs
                            nc.tensor.matmul(
                                pa[:],
                                v_sb[jt][:, h * DP1 : (h + 1) * DP1],
                                pts[jt][:, hs * 512 : (hs + 1) * 512],
                                start=(jt == 0),
                                stop=(jt == NT - 1),
                            )
                            if jt == NT - 1:
                                st = staged_pool.tile(
                                    [DP1, 512], f32, tag="staged", name="staged"
                                )
                                nc.vector.tensor_copy(st[:], pa[:])
                                if hs == 0:
                                    state["sums"] = sums_pool.tile(
                                        [2, 512], f32, tag="sums", name="sums"
                                    )
                                sums = state["sums"]
                                nc.sync.dma_start(
                                    sums[hs : hs + 1, :], st[D : D + 1, :]
                                )
                                state["stg"].append(st)
                                if hs == 1:
                                    # recip/recipb are consumed by the next
                                    # op in the same stream: 1 buf each
                                    recip = sums_pool.tile(
                                        [2, 512], f32, tag="recip", name="recip",
                                        bufs=1,
                                    )
                                    nc.vector.reciprocal_approx_fast(
                                        recip[:], sums[:]
                                    )
                                    rb = sums_pool.tile(
                                        [2, 512], bf16, tag="recipb", name="recipb",
                                        bufs=1,
                                    )
                                    nc.vector.tensor_copy(rb[:], recip[:])
                                    nc.sync.dma_start(rd[:], rb[:])
                                    for h2 in range(2):
                                        bc = bcast_pool.tile(
                                            [D, 512], bf16, tag="bcast", name="bcast"
                                        )
                                        # all broadcasts ride the sync HW
                                        # ring: at 64KB (bf16) each they
                                        # land in ~0.5us, while the gpsimd
                                        # software DGE runs multi-us late
                                        # and its lateness stalls fin muls
                                        # at the DVE FIFO head
                                        nc.sync.dma_start(
                                            bc[:],
                                            rd[h2 : h2 + 1, :].to_broadcast((D, 512)),
                                        )
                                        state["bcs"].append(bc)
                                    fin_box.append(
                                        (hp, ih, state["stg"], state["bcs"])
                                    )

                        return t

                    # hs1's first matmul reuses hs0's single pat buffer and
                    # so waits on hs0's DVE staged-copy; gating it to slot 5
                    # guarantees the copy (issued slot 3) lands first, else
                    # the PE idles 1.5-4us mid-half (and sometimes HAM
                    # re-throttles on top).
                    def mslot(hs, jt):
                        if self_mode:
                            return jt
                        # hp=7's chains feed the tail: start them ASAP
                        return 5 if (hs == 1 and jt == 0 and hp < 7) else 0

                    return [
                        (mslot(hs, jt), mk(hs, jt))
                        for hs in range(2)
                        for jt in range(NT)
                    ]

                def av_finish(fin):
                    hp, ih, stg, bcs = fin
                    for hs in range(2):
                        base = hs * 64
                        nc.vector.tensor_mul(
                            attnT[hp][ih][base : base + 64, :],
                            stg[hs][0:D, :],
                            bcs[hs][:],
                        )

                def v_loop_chain(m):
                    """V chain for token-tile m, heads 8..15, as 8 filler
                    thunks for half (0,0); accumulates in the pat pool
                    (idle until (0,1) since (0,0) has no AV work)."""
                    state = {}

                    def mk(k):
                        def t():
                            if k == 0:
                                state["ps"] = pat_pool.tile(
                                    [P, 512], f32, tag="pat", name="psvl"
                                )
                            ps = state["ps"]
                            nc.tensor.matmul(
                                ps[:],
                                xa[:, m * N + k * P : m * N + (k + 1) * P],
                                wva1[:, k * 512 : (k + 1) * 512],
                                start=(k == 0),
                                stop=(k == KE - 1),
                            )
                            if k == KE - 1:
                                src3 = ps[:].rearrange("p (h c) -> p h c", c=D)
                                dst3 = v_sb[m][:].rearrange(
                                    "p (h c) -> p h c", c=DP1
                                )
                                nc.vector.tensor_copy(dst3[:, 8:16, 0:D], src3)
                                nc.vector.memset(dst3[:, 8:16, D : D + 1], 1.0)

                        return t

                    return [mk(k) for k in range(KE)]

                def sc_half(hp, ih, av_work, qk_work, pts_live=None, fins=(),
                            v_work=None):
                    """One attention half: 4 double-slots, each issuing TWO
                    adjacent score-pairs (so the full-row <-> row-tiled
                    LDWEIGHTS transition tax is paid once per 2 slots), then
                    4 AV + 4 qkT filler matmuls (8 AV when no qkT).
                    Deferred attnT muls (fins) are emitted at double-slots
                    2/3."""
                    isl = slice(ih * 512, (ih + 1) * 512)
                    qt, kt = qkT[hp], qkT[8 + hp]
                    fins = list(fins)
                    v_work = v_work if v_work is not None else []
                    pts = pts_live if pts_live is not None else []
                    for jp in range(NT // 2):
                        for jt in (2 * jp, 2 * jp + 1):
                            ps = psum_pool.tile([P, N], f32, tag="big", name="psc")
                            jsl = slice(jt * P, (jt + 1) * P)
                            nc.tensor.matmul(
                                ps[:, 0:512], kt[0:64, jsl], qt[0:64, isl],
                                start=True, stop=True,
                            )
                            nc.tensor.matmul(
                                ps[:, 512:1024], kt[64:128, jsl], qt[64:128, isl],
                                start=True, stop=True,
                            )
                            pt = probs_pool.tile(
                                [P, N], bf16, tag="probs", name="probs"
                            )
                            nc.scalar.activation(pt[:], ps[:], AF.Exp)
                            pts.append(pt)
                        if fins and jp in (2, 3):
                            av_finish(fins.pop(0))
                        for jt in (2 * jp, 2 * jp + 1):
                            n_av = 2 if qk_work else 4
                            taken = 0
                            while av_work and taken < n_av and av_work[0][0] <= jt:
                                av_work.pop(0)[1]()
                                taken += 1
                            for _ in range(2):
                                if qk_work:
                                    qk_work.pop(0)[1]()
                            for _ in range(2):
                                if v_work:
                                    v_work.pop(0)()
                    while av_work:
                        av_work.pop(0)[1]()
                    while qk_work:
                        qk_work.pop(0)[1]()
                    while v_work:
                        v_work.pop(0)()
                    return pts

                def out_proj(ih, cs=range(4)):
                    for c in cs:
                        mi = ih * 4 + c
                        ps = psum_pool.tile([P, E], f32, tag="big", name="psy")
                        for nh in range(2):
                            nsl = slice(nh * 512, (nh + 1) * 512)
                            for k in range(KE):
                                nc.tensor.matmul(
                                    ps[:, nsl],
                                    attnT[k][ih][:, c * P : (c + 1) * P],
                                    woa[:, k * E + nh * 512 : k * E + (nh + 1) * 512],
                                    start=(k == 0),
                                    stop=(k == KE - 1),
                                )
                        for yh in range(2):
                            ysl = slice(yh * 512, (yh + 1) * 512)
                            ysb = y_pool.tile([P, 512], f32, tag="ysb", name="ysb")
                            nc.vector.tensor_add(ysb[:], ps[:, ysl], bias_bc[:, ysl])
                            nc.sync.dma_start(
                                y_d[mi * P : (mi + 1) * P, ysl], ysb[:]
                            )

                # Pipeline: half (hp, ih) runs the AV matmuls of the previous
                # half and the qkT matmuls of a tile needed 1-2 hp later
                # (kept per-half so psqk chains never straddle the
                # DVE-heavy half boundaries; the AV-less (0,0) half also
                # absorbs qkT[0]'s deferred nh1 half at 3 thunks/slot).
                # attnT muls (av_finish) are deferred one half so broadcast
                # DMA latency stays off the DVE critical path.
                fin_box = []
                fin_q = []
                pend_av = None  # (hp, ih, pts) awaiting AV matmuls
                for hp in range(8):
                    for ih in range(2):
                        av_work = (
                            av_thunks(*pend_av, fin_box) if pend_av else []
                        )
                        if hp < 7:
                            qk_work = qk_thunks(
                                (hp + 1) if ih == 0 else (8 + hp + 1)
                            )
                            pend_is_self = False
                        else:
                            # hp=7 has no qkT filler: ride this half's own AV
                            # matmuls inside it (with slot lag) so nothing
                            # trails the last half before out_proj.
                            qk_work = []
                            pts_live = []
                            av_work = av_work + av_thunks(
                                hp, ih, pts_live, fin_box, self_mode=True
                            )
                            pend_is_self = True
                        # defer fins an extra half for hp<6: the fin muls
                        # otherwise reach the strict-FIFO DVE queue head
                        # before their gpsimd broadcast lands, jamming the
                        # qkT bias-add behind them (-> PE stall -> HAM)
                        fins = []
                        while (
                            len(fin_q) > (2 if hp < 6 else 1) and len(fins) < 2
                        ):
                            fins.append(fin_q.pop(0))
                        v_work = None
                        if hp == 0 and ih == 0:
                            v_work = v_loop_chain(6) + v_loop_chain(7)
                        pts = sc_half(
                            hp,
                            ih,
                            av_work,
                            qk_work,
                            pts_live if pend_is_self else None,
                            fins,
                            v_work,
                        )
                        pend_av = None if pend_is_self else (hp, ih, pts)
                        while fin_box:
                            fin_q.append(fin_box.pop(0))
                # drain: remaining finishes, then the output projections;
                # out_proj(0)'s matmuls cover the (7,1) broadcast latency.
                while fin_q:
                    av_finish(fin_q.pop(0))
                out_proj(0)
                out_proj(1)

    nc.compile()
    return nc


def _prep_inputs(x, qkv_w, qkv_b, out_w, out_b):
    bf = ml_dtypes.bfloat16
    scale = np.float32(D ** -0.5)

    wq = (qkv_w[:E] * scale).astype(np.float32)
    wk = qkv_w[E : 2 * E]
    wv = qkv_w[2 * E :]
    wqkT = np.concatenate([wq, wk], axis=0).T.astype(bf)
    wvT = wv.T.astype(bf)
    woT = out_w.T.astype(bf)
    # Concatenated SBUF-ready layouts (one partition-dim-128 strip each):
    # wqb[p, m*1024+k*128+c] = wqkT[k*128+p, m*128+c]   (m-grouped)
    wqb = np.ascontiguousarray(
        wqkT.reshape(KE, P, MQK, P).transpose(1, 2, 0, 3).reshape(P, KE * 2 * E)
    )
    # wvb[p, nh*4096+k*512+e] = wvT[k*128+p, nh*512+e]  (nh-grouped)
    wvb = np.ascontiguousarray(
        wvT.reshape(KE, P, 2, 512).transpose(1, 2, 0, 3).reshape(P, KE * E)
    )
    # wob[p, k*1024+e] = woT[k*128+p, e]                (k-major)
    wob = np.ascontiguousarray(
        woT.reshape(KE, P, E).transpose(1, 0, 2).reshape(P, KE * E)
    )

    qkb = np.concatenate([qkv_b[:E] * scale, qkv_b[E : 2 * E]]).astype(np.float32)
    qkb = np.ascontiguousarray(qkb.reshape(MQK, P).T)
    # V-bias passes through softmax unchanged (rows sum to 1), so fold it
    # into the out-proj bias: y = attn0 @ Wo^T + (b_o + Wo @ b_v).
    vb = qkv_b[2 * E :].astype(np.float64)
    ob = (out_b.astype(np.float64) + out_w.astype(np.float64) @ vb).astype(
        np.float32
    ).reshape(1, E)

    in_maps = []
    for b in range(B):
        xT = x[b].T.astype(bf)
        # xb[p, m*1024+k*128+c] = xT[k*128+p, m*128+c]  (m-grouped)
        xb = np.ascontiguousarray(
            xT.reshape(KE, P, NT, P).transpose(1, 2, 0, 3).reshape(P, KE * N)
        )
        in_maps.append(
            {
                "xb": xb,
                "wqb": wqb,
                "qkb": qkb,
                "wvb": wvb,
                "wob": wob,
                "ob": ob,
            }
        )
    return in_maps


def _ensure_ntff_hook():
    import sys
    import types

    try:
        from antenv.axon_hooks import get_axon_ntff_profile_hook  # noqa: F401

        return
    except ImportError:
        pass
    try:
        from trn_agent_boot.trn_boot import _ntff_profile_via_ctypes

        hook = _ntff_profile_via_ctypes("/opt/axon/libaxon_pjrt.so")
    except Exception:
        hook = None
    mod = types.ModuleType("antenv.axon_hooks")
    mod.get_axon_ntff_profile_hook = lambda: hook
    sys.modules["antenv.axon_hooks"] = mod


def kernel(x, qkv_w, qkv_b, out_w, out_b):
    global _COMPILED, LAST_EXEC_NS, LAST_RESULT
    from concourse.bass_utils import run_bass_kernel_spmd

    if TRACE:
        _ensure_ntff_hook()

    if _COMPILED is None:
        _COMPILED = _build()
    nc = _COMPILED

    in_maps = _prep_inputs(
        np.asarray(x, np.float32),
        np.asarray(qkv_w, np.float32),
        np.asarray(qkv_b, np.float32),
        np.asarray(out_w, np.float32),
        np.asarray(out_b, np.float32),
    )

    res = run_bass_kernel_spmd(nc, in_maps, core_ids=list(range(B)), trace=TRACE)
    LAST_RESULT = res
    LAST_EXEC_NS = res.exec_time_ns

    y = np.stack([np.asarray(res.results[c]["y"]) for c in range(B)], axis=0)
    return y.astype(np.float32)

